# revision 15
# baseline (speedup 1.0000x reference)
"""TRN2 Bass kernel for nn_ClassAttention (1x1 conv + BN + ReLU + windowed attention).

kernel(**inputs) takes FULL inputs, returns the FULL output [4,256,256,256] f32.
Shards data-parallel over (batch, image-row-half) across 8 NeuronCores, runs a
Bass/Tile SPMD program via run_bass_kernel_spmd, and unshards on the host.

The kernel is DMA-bound (attn_i alone is 128 MiB/core in f32), so all wire
traffic is bf16: x and attn_i are converted host-side, the conv/attention
matmuls run in bf16 (f32 PSUM accumulation), and the output is staged bf16 and
upconverted host-side. Tolerated error budget is 2e-2; bf16 end-to-end lands
~1e-3.

Per-core shard (core = (b, rh) = (core//2, core%2)):
  x_sh   [256c, 16hh, 2048]   bf16, x[b,:,128rh:+128,:] window-contiguous:
                              [c, hh, (pw, win, r1, r2)]
  at_sh  [16hh, 128, 16384]   bf16, attn pre-transposed [pair, 64*win+k,
                              64*nh+q], partition-major per row of windows
  w_prep [256c, 256o]         bf16, (w_conv * inv_std[:,None]).T (BN folded)
  bias   [128, 1024]          f32, (beta - mean*inv_std) broadcast, 4x repeat
  out    [16hh, 128p, 4096]   bf16 staging dump; host decodes
                              p = 32quad+16win+d, f = pw*256+u*64+r1*8+r2,
                              ch = 64u+16quad+d, nh = 4u+quad

On-chip pipeline per window-pair (2 windows of 64 pixels, pixels on partitions):
  conv (PE): psum[128pix=(win,r1,r2), 256ch] = x_pair.T @ w_prep
             2 bf16 matmuls (K=128 halves), M=128, N=256; 4 pairs share one
             [128,1024] psum tile
  bias (DVE): tmp[128,2048] = psum + bias_tile, batched per 4-pair half-group
              (ACT/DVE cost ~ (N_per_lane + ~350cyc)/f, so batching amortizes
              the fixed per-instruction overhead)
  relu (ACT): block-diagonal V [128, (pair,nh,win,d)] bf16, batched per 8-pair
              group (2 instrs of N=2048/lane): diag cells = relu(tmp), off-diag
              cells stay zero (zeroed once at start, never rewritten)
  attn (PE): per head nh: one bf16 matmul computes BOTH windows via block-diag
             V: out[32,64] = V[:,32nh:+32].T @ At[:,64nh:+64], K=128, N=64,
             tile_position=(0, 32*(nh%4)) -> 4 column-tiles packed in the array
  evac (DVE): psum [128,(u,r1,r2)] f32 -> staging bf16 [128, 4096]
  store (HWDGE): staging -> DRAM, 1 MiB contiguous per row of windows
"""

import numpy as np
from contextlib import ExitStack

import ml_dtypes

import concourse.bacc as bacc
import concourse.tile as tile
import concourse.mybir as mybir
from concourse.bass_utils import run_bass_kernel_spmd

F32 = mybir.dt.float32
BF16 = mybir.dt.bfloat16
NP_BF16 = ml_dtypes.bfloat16
RELU = mybir.ActivationFunctionType.Relu

EPS = 1e-5
NCORES = 8

_cached_nc = None


def _build_program(n_vbd=3, at_bufs=4, G=8):
    nc = bacc.Bacc("TRN2", target_bir_lowering=False, debug=False)

    x_d = nc.dram_tensor("x_sh", [256, 16, 2048], BF16, kind="ExternalInput")
    at_d = nc.dram_tensor("at_sh", [16, 128, 16384], BF16, kind="ExternalInput")
    wc_d = nc.dram_tensor("w_prep", [256, 256], BF16, kind="ExternalInput")
    b_d = nc.dram_tensor("bias", [128, 1024], F32, kind="ExternalInput")
    out_d = nc.dram_tensor("out_sh", [16, 128, 4096], BF16, kind="ExternalOutput")

    ngroups = 16 // G
    HG = G // 2  # pairs per half-group (one conv psum tile)

    with tile.TileContext(nc) as tc, ExitStack() as ctx:
        const = ctx.enter_context(tc.tile_pool(name="const", bufs=1))
        xp = ctx.enter_context(tc.tile_pool(name="xp", bufs=2))
        atp = ctx.enter_context(tc.tile_pool(name="atp", bufs=at_bufs))
        vbdp = ctx.enter_context(tc.tile_pool(name="vbdp", bufs=1))
        tvp = ctx.enter_context(tc.tile_pool(name="tvp", bufs=2))
        stp = ctx.enter_context(tc.tile_pool(name="stp", bufs=2))
        pscp = ctx.enter_context(tc.tile_pool(name="pscp", bufs=2, space="PSUM"))
        psap = ctx.enter_context(tc.tile_pool(name="psap", bufs=4, space="PSUM"))

        w0 = const.tile([128, 256], BF16, name="w0")
        w1 = const.tile([128, 256], BF16, name="w1")
        nc.scalar.dma_start(out=w0, in_=wc_d[0:128, :])
        nc.scalar.dma_start(out=w1, in_=wc_d[128:256, :])
        bias = const.tile([128, 1024], F32, name="bias_t")
        nc.scalar.dma_start(out=bias, in_=b_d[:, :])

        # Block-diagonal V tiles, one per G-pair group: columns =
        # (pair G, nh 16, two 2, d 16). Zeroed once; the batched relu writes
        # only the diagonal cells (win0 -> rows 0:64 of win-0 columns, win1 ->
        # rows 64:128 of win-1 columns), so the zeros persist across reuse and
        # each V[:, 512*iG+32*nh:+32] is exactly block-diag(V0, V1).
        vbd = []
        for i in range(n_vbd):
            t = vbdp.tile([128, 512 * G], BF16, tag=f"vbd{i}", name=f"vbd{i}")
            nc.vector.memset(t, 0.0)
            vbd.append(t)
        vbd_i = 0

        for hh in range(16):
            xt0 = xp.tile([128, 2048], BF16, tag="xt0", name=f"xt0_{hh}")
            xt1 = xp.tile([128, 2048], BF16, tag="xt1", name=f"xt1_{hh}")
            nc.scalar.dma_start(out=xt0, in_=x_d[0:128, hh, :])
            nc.scalar.dma_start(out=xt1, in_=x_d[128:256, hh, :])

            st = stp.tile([128, 4096], BF16, tag="st", name=f"st_{hh}")
            # f = pw*256 + u*64 + r1*8 + r2 (pair-major: evac writes are
            # contiguous and the store can be issued in quarters)
            st_r = st.rearrange("p (pw u r1 r2) -> p pw u r1 r2",
                                u=4, r1=8, pw=16, r2=8)

            for g in range(ngroups):
                at = atp.tile([128, 1024 * G], BF16, tag="at", name=f"at_{hh}_{g}")
                # 3:1 sync:scalar split keeps queue bytes balanced (x, bias
                # and out stores ride the scalar queue)
                dma_eng = nc.scalar if (hh * ngroups + g) % 4 == 3 else nc.sync
                dma_eng.dma_start(
                    out=at,
                    in_=at_d[hh, :, 1024 * G * g: 1024 * G * (g + 1)])

                # conv + bias for the group, batched per half-group
                tvw = tvp.tile([128, 256 * G], F32, tag="tv", name=f"tv_{hh}_{g}")
                for h in range(2):
                    ps = pscp.tile([128, 256 * HG], F32, tag="psc",
                                   name=f"ps_{hh}_{g}_{h}")
                    for i in range(HG):
                        p8 = G * g + HG * h + i
                        xsl = slice(128 * p8, 128 * p8 + 128)
                        dst = ps[:, 256 * i:256 * i + 256]
                        nc.tensor.matmul(dst, xt0[:, xsl], w0,
                                         start=True, stop=False)
                        nc.tensor.matmul(dst, xt1[:, xsl], w1,
                                         start=False, stop=True)
                    nc.vector.tensor_add(
                        tvw[:, 256 * HG * h:256 * HG * (h + 1)], ps, bias)

                # relu into block-diag V, batched over the whole group
                V = vbd[vbd_i % n_vbd]
                vbd_i += 1
                Vw = V.rearrange("p (pr nh two d) -> p (pr nh) two d",
                                 pr=G, nh=16, two=2, d=16)
                tvr = tvw.rearrange("p (a b) -> p a b", a=16 * G)
                nc.scalar.activation(Vw[0:64, :, 0, :], tvr[0:64], RELU)
                nc.scalar.activation(Vw[64:128, :, 1, :], tvr[64:128], RELU)

                for iG in range(G):
                    p8 = G * g + iG
                    pa = psap.tile([128, 256], F32, tag="pa", name=f"pa_{hh}_{p8}")
                    for j in range(4):
                        for quad in range(4):
                            nh = 4 * j + quad
                            nc.tensor.matmul(
                                pa[32 * quad:32 * quad + 32, 64 * j:64 * j + 64],
                                V[:, 512 * iG + 32 * nh:512 * iG + 32 * nh + 32],
                                at[:, 1024 * iG + 64 * nh: 1024 * iG + 64 * nh + 64],
                                start=True, stop=True,
                                tile_position=(0, 32 * quad))
                    src = pa.rearrange("p (u r1 r2) -> p u r1 r2", u=4, r1=8, r2=8)
                    nc.vector.tensor_copy(st_r[:, p8], src)
                    # store finished quarters early so the tail drains fast
                    if p8 % 4 == 3:
                        q4 = p8 // 4
                        nc.scalar.dma_start(
                            out=out_d[hh, :, 1024 * q4:1024 * (q4 + 1)],
                            in_=st[:, 1024 * q4:1024 * (q4 + 1)])

    nc.compile()
    return nc


def _shard_inputs(x, attn_i, w_conv, bn_gamma, bn_beta, bn_mean, bn_var):
    inv_std = (bn_gamma / np.sqrt(bn_var + np.float32(EPS))).astype(np.float32)
    shift = (bn_beta - bn_mean * inv_std).astype(np.float32)
    bias_tile = np.ascontiguousarray(
        np.broadcast_to(shift[None, None, :], (128, 4, 256))
    ).reshape(128, 1024).astype(np.float32)
    w_prep = np.ascontiguousarray(
        (w_conv * inv_std[:, None]).T).astype(NP_BF16)
    x16 = np.asarray(x, dtype=NP_BF16)
    a16 = np.asarray(attn_i, dtype=NP_BF16)
    in_maps = []
    for core in range(NCORES):
        b, rh = core // 2, core % 2
        x_sh = x16[b, :, 128 * rh:128 * rh + 128, :]
        x_sh = np.ascontiguousarray(
            x_sh.reshape(256, 16, 8, 16, 2, 8).transpose(0, 1, 3, 4, 2, 5)
        ).reshape(256, 16, 2048)
        a_sl = a16[1024 * b + 512 * rh: 1024 * b + 512 * rh + 512]
        # [pair, 64win+k, 64nh+q], then partition-major per hh row
        # ([hh, p, pr, 1024]) so each at-load reads contiguous per partition
        a_prep = a_sl.reshape(256, 2, 16, 64, 64).transpose(0, 1, 4, 2, 3) \
            .reshape(16, 16, 128, 1024)
        a_prep = np.ascontiguousarray(
            a_prep.transpose(0, 2, 1, 3)).reshape(16, 128, 16384)
        in_maps.append(dict(x_sh=x_sh, at_sh=a_prep, w_prep=w_prep, bias=bias_tile))
    return in_maps


def _unshard_output(results):
    out = np.empty((4, 256, 256, 256), np.float32)
    for core in range(NCORES):
        b, rh = core // 2, core % 2
        raw = np.asarray(results[core]["out_sh"]).astype(np.float32)
        r = raw.reshape(16, 4, 2, 16, 16, 4, 8, 8)  # hh,quad,win,d,pw,u,r1,r2
        # ch = 64u+16quad+d ; h = 8hh+r1 ; w = 16pw+8win+r2
        oc = r.transpose(5, 1, 3, 0, 6, 4, 2, 7).reshape(256, 128, 256)
        out[b, :, 128 * rh:128 * rh + 128, :] = oc
    return out


def get_program():
    global _cached_nc
    if _cached_nc is None:
        _cached_nc = _build_program()
    return _cached_nc


def run_sharded(in_maps, trace=False, **kwargs):
    nc = get_program()
    return run_bass_kernel_spmd(nc, in_maps, list(range(NCORES)),
                                trace=trace, **kwargs)


def kernel(x, attn_i, w_conv, bn_gamma, bn_beta, bn_mean, bn_var):
    x = np.asarray(x, dtype=np.float32)
    attn_i = np.asarray(attn_i, dtype=np.float32)
    w_conv = np.asarray(w_conv, dtype=np.float32)
    bn_gamma = np.asarray(bn_gamma, dtype=np.float32)
    bn_beta = np.asarray(bn_beta, dtype=np.float32)
    bn_mean = np.asarray(bn_mean, dtype=np.float32)
    bn_var = np.asarray(bn_var, dtype=np.float32)
    in_maps = _shard_inputs(x, attn_i, w_conv, bn_gamma, bn_beta, bn_mean, bn_var)
    res = run_sharded(in_maps)
    return _unshard_output(res.results)


# revision 19
# speedup vs baseline: 1.2518x; 1.2518x over previous
"""TRN2 Bass kernel for nn_ClassAttention (1x1 conv + BN + ReLU + windowed attention).

kernel(**inputs) takes FULL inputs, returns the FULL output [4,256,256,256] f32.
Shards data-parallel over (batch, image-row-half) across 8 NeuronCores, runs a
Bass/Tile SPMD program via run_bass_kernel_spmd, and unshards on the host.

The kernel is DMA-bound (attn_i alone is 128 MiB/core in f32), so all wire
traffic is bf16: x and attn_i are converted host-side, the conv/attention
matmuls run in bf16 (f32 PSUM accumulation), and the output is staged bf16 and
upconverted host-side. Tolerated error budget is 2e-2; bf16 end-to-end lands
~1e-3.

Per-core shard (core = (b, rh) = (core//2, core%2)):
  x_sh   [256c, 16hh, 2048]   bf16, x[b,:,128rh:+128,:] window-contiguous:
                              [c, hh, (pw, win, r1, r2)]
  at_sh  [16hh, 128, 16384]   bf16, attn pre-transposed [pair, 64*win+k,
                              64*nh+q], partition-major per row of windows
  w_prep [256c, 256o]         bf16, (w_conv * inv_std[:,None]).T (BN folded)
  bias   [128, 1024]          f32, (beta - mean*inv_std) broadcast, 4x repeat
  out    [16hh, 128p, 4096]   bf16 staging dump; host decodes
                              p = 32quad+16win+d, f = pw*256+u*64+r1*8+r2,
                              ch = 64u+16quad+d, nh = 4u+quad

On-chip pipeline per window-pair (2 windows of 64 pixels, pixels on partitions):
  conv (PE): psum[128pix=(win,r1,r2), 256ch] = x_pair.T @ w_prep
             2 bf16 matmuls (K=128 halves), M=128, N=256; 4 pairs share one
             [128,1024] psum tile
  bias (DVE): tmp[128,2048] = psum + bias_tile, batched per 4-pair half-group
              (ACT/DVE cost ~ (N_per_lane + ~350cyc)/f, so batching amortizes
              the fixed per-instruction overhead)
  relu (ACT): block-diagonal V [128, (pair,nh,win,d)] bf16, batched per 8-pair
              group (2 instrs of N=2048/lane): diag cells = relu(tmp), off-diag
              cells stay zero (zeroed once at start, never rewritten)
  attn (PE): per head nh: one bf16 matmul computes BOTH windows via block-diag
             V: out[32,64] = V[:,32nh:+32].T @ At[:,64nh:+64], K=128, N=64,
             tile_position=(0, 32*(nh%4)) -> 4 column-tiles packed in the array
  evac (DVE): psum [128,(u,r1,r2)] f32 -> staging bf16 [128, 4096]
  store (HWDGE): staging -> DRAM, 1 MiB contiguous per row of windows
"""

import numpy as np
from contextlib import ExitStack

import ml_dtypes

import concourse.bacc as bacc
import concourse.tile as tile
import concourse.mybir as mybir
from concourse.bass_utils import run_bass_kernel_spmd

F32 = mybir.dt.float32
BF16 = mybir.dt.bfloat16
NP_BF16 = ml_dtypes.bfloat16
RELU = mybir.ActivationFunctionType.Relu

EPS = 1e-5
NCORES = 8

_cached_nc = None


def _build_program(n_vbd=3, at_bufs=4, G=8):
    nc = bacc.Bacc("TRN2", target_bir_lowering=False, debug=False)

    x_d = nc.dram_tensor("x_sh", [256, 16, 2048], BF16, kind="ExternalInput")
    at_d = nc.dram_tensor("at_sh", [16, 128, 16384], BF16, kind="ExternalInput")
    wc_d = nc.dram_tensor("w_prep", [256, 256], BF16, kind="ExternalInput")
    b_d = nc.dram_tensor("bias", [128, 1024], F32, kind="ExternalInput")
    out_d = nc.dram_tensor("out_sh", [16, 128, 4096], BF16, kind="ExternalOutput")

    ngroups = 16 // G
    HG = G // 2  # pairs per half-group (one conv psum tile)

    with tile.TileContext(nc) as tc, ExitStack() as ctx:
        const = ctx.enter_context(tc.tile_pool(name="const", bufs=1))
        xp = ctx.enter_context(tc.tile_pool(name="xp", bufs=2))
        atp = ctx.enter_context(tc.tile_pool(name="atp", bufs=at_bufs))
        vbdp = ctx.enter_context(tc.tile_pool(name="vbdp", bufs=1))
        tvp = ctx.enter_context(tc.tile_pool(name="tvp", bufs=2))
        stp = ctx.enter_context(tc.tile_pool(name="stp", bufs=2))
        pscp = ctx.enter_context(tc.tile_pool(name="pscp", bufs=2, space="PSUM"))
        psap = ctx.enter_context(tc.tile_pool(name="psap", bufs=4, space="PSUM"))

        w0 = const.tile([128, 256], BF16, name="w0")
        w1 = const.tile([128, 256], BF16, name="w1")
        nc.sync.dma_start(out=w0, in_=wc_d[0:128, :])
        nc.sync.dma_start(out=w1, in_=wc_d[128:256, :])
        bias = const.tile([128, 1024], F32, name="bias_t")
        nc.sync.dma_start(out=bias, in_=b_d[:, :])

        # Block-diagonal V tiles, one per G-pair group: columns =
        # (pair G, nh 16, two 2, d 16). Zeroed once; the batched relu writes
        # only the diagonal cells (win0 -> rows 0:64 of win-0 columns, win1 ->
        # rows 64:128 of win-1 columns), so the zeros persist across reuse and
        # each V[:, 512*iG+32*nh:+32] is exactly block-diag(V0, V1).
        vbd = []
        for i in range(n_vbd):
            t = vbdp.tile([128, 512 * G], BF16, tag=f"vbd{i}", name=f"vbd{i}")
            nc.vector.memset(t, 0.0)
            vbd.append(t)
        vbd_i = 0

        for hh in range(16):
            xt0 = xp.tile([128, 2048], BF16, tag="xt0", name=f"xt0_{hh}")
            xt1 = xp.tile([128, 2048], BF16, tag="xt1", name=f"xt1_{hh}")
            nc.sync.dma_start(out=xt0, in_=x_d[0:128, hh, :])
            nc.sync.dma_start(out=xt1, in_=x_d[128:256, hh, :])

            st = stp.tile([128, 4096], BF16, tag="st", name=f"st_{hh}")
            # f = pw*256 + u*64 + r1*8 + r2 (pair-major: evac writes are
            # contiguous and the store can be issued in quarters)
            st_r = st.rearrange("p (pw u r1 r2) -> p pw u r1 r2",
                                u=4, r1=8, pw=16, r2=8)

            for g in range(ngroups):
                at = atp.tile([128, 1024 * G], BF16, tag="at", name=f"at_{hh}_{g}")
                # alternate input queues to spread HWDGE load
                dma_eng = nc.sync if (hh * ngroups + g) % 2 == 0 else nc.scalar
                dma_eng.dma_start(
                    out=at,
                    in_=at_d[hh, :, 1024 * G * g: 1024 * G * (g + 1)])

                # conv + bias for the group, batched per half-group
                tvw = tvp.tile([128, 256 * G], F32, tag="tv", name=f"tv_{hh}_{g}")
                for h in range(2):
                    ps = pscp.tile([128, 256 * HG], F32, tag="psc",
                                   name=f"ps_{hh}_{g}_{h}")
                    for i in range(HG):
                        p8 = G * g + HG * h + i
                        xsl = slice(128 * p8, 128 * p8 + 128)
                        dst = ps[:, 256 * i:256 * i + 256]
                        nc.tensor.matmul(dst, xt0[:, xsl], w0,
                                         start=True, stop=False)
                        nc.tensor.matmul(dst, xt1[:, xsl], w1,
                                         start=False, stop=True)
                    nc.vector.tensor_add(
                        tvw[:, 256 * HG * h:256 * HG * (h + 1)], ps, bias)

                # relu into block-diag V, batched over the whole group
                V = vbd[vbd_i % n_vbd]
                vbd_i += 1
                Vw = V.rearrange("p (pr nh two d) -> p (pr nh) two d",
                                 pr=G, nh=16, two=2, d=16)
                tvr = tvw.rearrange("p (a b) -> p a b", a=16 * G)
                nc.scalar.activation(Vw[0:64, :, 0, :], tvr[0:64], RELU)
                nc.scalar.activation(Vw[64:128, :, 1, :], tvr[64:128], RELU)

                for iG in range(G):
                    p8 = G * g + iG
                    pa = psap.tile([128, 256], F32, tag="pa", name=f"pa_{hh}_{p8}")
                    for j in range(4):
                        for quad in range(4):
                            nh = 4 * j + quad
                            nc.tensor.matmul(
                                pa[32 * quad:32 * quad + 32, 64 * j:64 * j + 64],
                                V[:, 512 * iG + 32 * nh:512 * iG + 32 * nh + 32],
                                at[:, 1024 * iG + 64 * nh: 1024 * iG + 64 * nh + 64],
                                start=True, stop=True,
                                tile_position=(0, 32 * quad))
                    src = pa.rearrange("p (u r1 r2) -> p u r1 r2", u=4, r1=8, r2=8)
                    nc.vector.tensor_copy(st_r[:, p8], src)
                    # store finished halves early so the tail drains fast
                    if p8 % 8 == 7:
                        q2 = p8 // 8
                        nc.scalar.dma_start(
                            out=out_d[hh, :, 2048 * q2:2048 * (q2 + 1)],
                            in_=st[:, 2048 * q2:2048 * (q2 + 1)])

    nc.compile()
    return nc


def _shard_inputs(x, attn_i, w_conv, bn_gamma, bn_beta, bn_mean, bn_var):
    inv_std = (bn_gamma / np.sqrt(bn_var + np.float32(EPS))).astype(np.float32)
    shift = (bn_beta - bn_mean * inv_std).astype(np.float32)
    bias_tile = np.ascontiguousarray(
        np.broadcast_to(shift[None, None, :], (128, 4, 256))
    ).reshape(128, 1024).astype(np.float32)
    w_prep = np.ascontiguousarray(
        (w_conv * inv_std[:, None]).T).astype(NP_BF16)
    x16 = np.asarray(x, dtype=NP_BF16)
    a16 = np.asarray(attn_i, dtype=NP_BF16)
    in_maps = []
    for core in range(NCORES):
        b, rh = core // 2, core % 2
        x_sh = x16[b, :, 128 * rh:128 * rh + 128, :]
        x_sh = np.ascontiguousarray(
            x_sh.reshape(256, 16, 8, 16, 2, 8).transpose(0, 1, 3, 4, 2, 5)
        ).reshape(256, 16, 2048)
        a_sl = a16[1024 * b + 512 * rh: 1024 * b + 512 * rh + 512]
        # [pair, 64win+k, 64nh+q], then partition-major per hh row
        # ([hh, p, pr, 1024]) so each at-load reads contiguous per partition
        a_prep = a_sl.reshape(256, 2, 16, 64, 64).transpose(0, 1, 4, 2, 3) \
            .reshape(16, 16, 128, 1024)
        a_prep = np.ascontiguousarray(
            a_prep.transpose(0, 2, 1, 3)).reshape(16, 128, 16384)
        in_maps.append(dict(x_sh=x_sh, at_sh=a_prep, w_prep=w_prep, bias=bias_tile))
    return in_maps


def _unshard_output(results):
    out = np.empty((4, 256, 256, 256), np.float32)
    for core in range(NCORES):
        b, rh = core // 2, core % 2
        raw = np.asarray(results[core]["out_sh"]).astype(np.float32)
        r = raw.reshape(16, 4, 2, 16, 16, 4, 8, 8)  # hh,quad,win,d,pw,u,r1,r2
        # ch = 64u+16quad+d ; h = 8hh+r1 ; w = 16pw+8win+r2
        oc = r.transpose(5, 1, 3, 0, 6, 4, 2, 7).reshape(256, 128, 256)
        out[b, :, 128 * rh:128 * rh + 128, :] = oc
    return out


def get_program():
    global _cached_nc
    if _cached_nc is None:
        _cached_nc = _build_program()
    return _cached_nc


def run_sharded(in_maps, trace=False, **kwargs):
    nc = get_program()
    return run_bass_kernel_spmd(nc, in_maps, list(range(NCORES)),
                                trace=trace, **kwargs)


def kernel(x, attn_i, w_conv, bn_gamma, bn_beta, bn_mean, bn_var):
    x = np.asarray(x, dtype=np.float32)
    attn_i = np.asarray(attn_i, dtype=np.float32)
    w_conv = np.asarray(w_conv, dtype=np.float32)
    bn_gamma = np.asarray(bn_gamma, dtype=np.float32)
    bn_beta = np.asarray(bn_beta, dtype=np.float32)
    bn_mean = np.asarray(bn_mean, dtype=np.float32)
    bn_var = np.asarray(bn_var, dtype=np.float32)
    in_maps = _shard_inputs(x, attn_i, w_conv, bn_gamma, bn_beta, bn_mean, bn_var)
    res = run_sharded(in_maps)
    return _unshard_output(res.results)


# revision 20
# speedup vs baseline: 1.3159x; 1.0512x over previous
"""TRN2 Bass kernel for nn_ClassAttention (1x1 conv + BN + ReLU + windowed attention).

kernel(**inputs) takes FULL inputs, returns the FULL output [4,256,256,256] f32.
Shards data-parallel over (batch, image-row-half) across 8 NeuronCores, runs a
Bass/Tile SPMD program via run_bass_kernel_spmd, and unshards on the host.

The kernel is DMA-bound (attn_i alone is 128 MiB/core in f32), so all wire
traffic is bf16: x and attn_i are converted host-side, the conv/attention
matmuls run in bf16 (f32 PSUM accumulation), and the output is staged bf16 and
upconverted host-side. Tolerated error budget is 2e-2; bf16 end-to-end lands
~1e-3.

Per-core shard (core = (b, rh) = (core//2, core%2)):
  x_sh   [256c, 16hh, 2048]   bf16, x[b,:,128rh:+128,:] window-contiguous:
                              [c, hh, (pw, win, r1, r2)]
  at_sh  [16hh, 128, 16384]   bf16, attn pre-transposed [pair, 64*win+k,
                              64*nh+q], partition-major per row of windows
  w_prep [256c, 256o]         bf16, (w_conv * inv_std[:,None]).T (BN folded)
  bias   [128, 1024]          f32, (beta - mean*inv_std) broadcast, 4x repeat
  out    [16hh, 128p, 4096]   bf16 staging dump; host decodes
                              p = 32quad+16win+d, f = pw*256+u*64+r1*8+r2,
                              ch = 64u+16quad+d, nh = 4u+quad

On-chip pipeline per window-pair (2 windows of 64 pixels, pixels on partitions):
  conv (PE): psum[128pix=(win,r1,r2), 256ch] = x_pair.T @ w_prep
             2 bf16 matmuls (K=128 halves), M=128, N=256; 4 pairs share one
             [128,1024] psum tile
  bias (DVE): tmp[128,2048] = psum + bias_tile, batched per 4-pair half-group
              (ACT/DVE cost ~ (N_per_lane + ~350cyc)/f, so batching amortizes
              the fixed per-instruction overhead)
  relu (ACT): block-diagonal V [128, (pair,nh,win,d)] bf16, batched per 8-pair
              group (2 instrs of N=2048/lane): diag cells = relu(tmp), off-diag
              cells stay zero (zeroed once at start, never rewritten)
  attn (PE): per head nh: one bf16 matmul computes BOTH windows via block-diag
             V: out[32,64] = V[:,32nh:+32].T @ At[:,64nh:+64], K=128, N=64,
             tile_position=(0, 32*(nh%4)) -> 4 column-tiles packed in the array
  evac (DVE): psum [128,(u,r1,r2)] f32 -> staging bf16 [128, 4096]
  store (HWDGE): staging -> DRAM, 1 MiB contiguous per row of windows
"""

import numpy as np
from contextlib import ExitStack

import ml_dtypes

import concourse.bacc as bacc
import concourse.tile as tile
import concourse.mybir as mybir
from concourse.bass_utils import run_bass_kernel_spmd

F32 = mybir.dt.float32
BF16 = mybir.dt.bfloat16
NP_BF16 = ml_dtypes.bfloat16
RELU = mybir.ActivationFunctionType.Relu

EPS = 1e-5
NCORES = 8

_cached_nc = None


def _build_program(n_vbd=3, at_bufs=4, G=8):
    nc = bacc.Bacc("TRN2", target_bir_lowering=False, debug=False)

    x_d = nc.dram_tensor("x_sh", [256, 16, 2048], BF16, kind="ExternalInput")
    at_d = nc.dram_tensor("at_sh", [16, 128, 16384], BF16, kind="ExternalInput")
    wc_d = nc.dram_tensor("w_prep", [256, 256], BF16, kind="ExternalInput")
    b_d = nc.dram_tensor("bias", [128, 1024], F32, kind="ExternalInput")
    out_d = nc.dram_tensor("out_sh", [16, 128, 4096], BF16, kind="ExternalOutput")

    ngroups = 16 // G
    HG = G // 2  # pairs per half-group (one conv psum tile)

    with tile.TileContext(nc) as tc, ExitStack() as ctx:
        const = ctx.enter_context(tc.tile_pool(name="const", bufs=1))
        xp = ctx.enter_context(tc.tile_pool(name="xp", bufs=2))
        atp = ctx.enter_context(tc.tile_pool(name="atp", bufs=at_bufs))
        vbdp = ctx.enter_context(tc.tile_pool(name="vbdp", bufs=1))
        tvp = ctx.enter_context(tc.tile_pool(name="tvp", bufs=2))
        stp = ctx.enter_context(tc.tile_pool(name="stp", bufs=2))
        pscp = ctx.enter_context(tc.tile_pool(name="pscp", bufs=2, space="PSUM"))
        psap = ctx.enter_context(tc.tile_pool(name="psap", bufs=4, space="PSUM"))

        w0 = const.tile([128, 256], BF16, name="w0")
        w1 = const.tile([128, 256], BF16, name="w1")
        nc.sync.dma_start(out=w0, in_=wc_d[0:128, :])
        nc.sync.dma_start(out=w1, in_=wc_d[128:256, :])
        bias = const.tile([128, 1024], F32, name="bias_t")
        nc.sync.dma_start(out=bias, in_=b_d[:, :])

        # Block-diagonal V tiles, one per G-pair group: columns =
        # (pair G, nh 16, two 2, d 16). Zeroed once; the batched relu writes
        # only the diagonal cells (win0 -> rows 0:64 of win-0 columns, win1 ->
        # rows 64:128 of win-1 columns), so the zeros persist across reuse and
        # each V[:, 512*iG+32*nh:+32] is exactly block-diag(V0, V1).
        vbd = []
        for i in range(n_vbd):
            t = vbdp.tile([128, 512 * G], BF16, tag=f"vbd{i}", name=f"vbd{i}")
            nc.vector.memset(t, 0.0)
            vbd.append(t)
        vbd_i = 0

        def emit_attn(task):
            """Attention matmuls + evac + store for one group (deferred one
            group so the in-order PE queue never stalls on the relu chain)."""
            at, V, hh, g, st, st_r = task
            for iG in range(G):
                p8 = G * g + iG
                pa = psap.tile([128, 256], F32, tag="pa", name=f"pa_{hh}_{p8}")
                for j in range(4):
                    for quad in range(4):
                        nh = 4 * j + quad
                        nc.tensor.matmul(
                            pa[32 * quad:32 * quad + 32, 64 * j:64 * j + 64],
                            V[:, 512 * iG + 32 * nh:512 * iG + 32 * nh + 32],
                            at[:, 1024 * iG + 64 * nh: 1024 * iG + 64 * nh + 64],
                            start=True, stop=True,
                            tile_position=(0, 32 * quad))
                src = pa.rearrange("p (u r1 r2) -> p u r1 r2", u=4, r1=8, r2=8)
                nc.vector.tensor_copy(st_r[:, p8], src)
                # store finished halves early so the tail drains fast
                if p8 % 8 == 7:
                    q2 = p8 // 8
                    nc.scalar.dma_start(
                        out=out_d[hh, :, 2048 * q2:2048 * (q2 + 1)],
                        in_=st[:, 2048 * q2:2048 * (q2 + 1)])

        pending = None
        for hh in range(16):
            xt0 = xp.tile([128, 2048], BF16, tag="xt0", name=f"xt0_{hh}")
            xt1 = xp.tile([128, 2048], BF16, tag="xt1", name=f"xt1_{hh}")
            nc.sync.dma_start(out=xt0, in_=x_d[0:128, hh, :])
            nc.sync.dma_start(out=xt1, in_=x_d[128:256, hh, :])

            st = stp.tile([128, 4096], BF16, tag="st", name=f"st_{hh}")
            # f = pw*256 + u*64 + r1*8 + r2 (pair-major: evac writes are
            # contiguous and the store can be issued in halves)
            st_r = st.rearrange("p (pw u r1 r2) -> p pw u r1 r2",
                                u=4, r1=8, pw=16, r2=8)

            for g in range(ngroups):
                at = atp.tile([128, 1024 * G], BF16, tag="at", name=f"at_{hh}_{g}")
                # alternate input queues to spread HWDGE load
                dma_eng = nc.sync if (hh * ngroups + g) % 2 == 0 else nc.scalar
                dma_eng.dma_start(
                    out=at,
                    in_=at_d[hh, :, 1024 * G * g: 1024 * G * (g + 1)])

                # conv + bias for the group, batched per half-group
                tvw = tvp.tile([128, 256 * G], F32, tag="tv", name=f"tv_{hh}_{g}")
                for h in range(2):
                    ps = pscp.tile([128, 256 * HG], F32, tag="psc",
                                   name=f"ps_{hh}_{g}_{h}")
                    for i in range(HG):
                        p8 = G * g + HG * h + i
                        xsl = slice(128 * p8, 128 * p8 + 128)
                        dst = ps[:, 256 * i:256 * i + 256]
                        nc.tensor.matmul(dst, xt0[:, xsl], w0,
                                         start=True, stop=False)
                        nc.tensor.matmul(dst, xt1[:, xsl], w1,
                                         start=False, stop=True)
                    nc.vector.tensor_add(
                        tvw[:, 256 * HG * h:256 * HG * (h + 1)], ps, bias)

                # relu into block-diag V, batched over the whole group
                V = vbd[vbd_i % n_vbd]
                vbd_i += 1
                Vw = V.rearrange("p (pr nh two d) -> p (pr nh) two d",
                                 pr=G, nh=16, two=2, d=16)
                tvr = tvw.rearrange("p (a b) -> p a b", a=16 * G)
                nc.scalar.activation(Vw[0:64, :, 0, :], tvr[0:64], RELU)
                nc.scalar.activation(Vw[64:128, :, 1, :], tvr[64:128], RELU)

                if pending is not None:
                    emit_attn(pending)
                pending = (at, V, hh, g, st, st_r)

        emit_attn(pending)

    nc.compile()
    return nc


def _shard_inputs(x, attn_i, w_conv, bn_gamma, bn_beta, bn_mean, bn_var):
    inv_std = (bn_gamma / np.sqrt(bn_var + np.float32(EPS))).astype(np.float32)
    shift = (bn_beta - bn_mean * inv_std).astype(np.float32)
    bias_tile = np.ascontiguousarray(
        np.broadcast_to(shift[None, None, :], (128, 4, 256))
    ).reshape(128, 1024).astype(np.float32)
    w_prep = np.ascontiguousarray(
        (w_conv * inv_std[:, None]).T).astype(NP_BF16)
    x16 = np.asarray(x, dtype=NP_BF16)
    a16 = np.asarray(attn_i, dtype=NP_BF16)
    in_maps = []
    for core in range(NCORES):
        b, rh = core // 2, core % 2
        x_sh = x16[b, :, 128 * rh:128 * rh + 128, :]
        x_sh = np.ascontiguousarray(
            x_sh.reshape(256, 16, 8, 16, 2, 8).transpose(0, 1, 3, 4, 2, 5)
        ).reshape(256, 16, 2048)
        a_sl = a16[1024 * b + 512 * rh: 1024 * b + 512 * rh + 512]
        # [pair, 64win+k, 64nh+q], then partition-major per hh row
        # ([hh, p, pr, 1024]) so each at-load reads contiguous per partition
        a_prep = a_sl.reshape(256, 2, 16, 64, 64).transpose(0, 1, 4, 2, 3) \
            .reshape(16, 16, 128, 1024)
        a_prep = np.ascontiguousarray(
            a_prep.transpose(0, 2, 1, 3)).reshape(16, 128, 16384)
        in_maps.append(dict(x_sh=x_sh, at_sh=a_prep, w_prep=w_prep, bias=bias_tile))
    return in_maps


def _unshard_output(results):
    out = np.empty((4, 256, 256, 256), np.float32)
    for core in range(NCORES):
        b, rh = core // 2, core % 2
        raw = np.asarray(results[core]["out_sh"]).astype(np.float32)
        r = raw.reshape(16, 4, 2, 16, 16, 4, 8, 8)  # hh,quad,win,d,pw,u,r1,r2
        # ch = 64u+16quad+d ; h = 8hh+r1 ; w = 16pw+8win+r2
        oc = r.transpose(5, 1, 3, 0, 6, 4, 2, 7).reshape(256, 128, 256)
        out[b, :, 128 * rh:128 * rh + 128, :] = oc
    return out


def get_program():
    global _cached_nc
    if _cached_nc is None:
        _cached_nc = _build_program()
    return _cached_nc


def run_sharded(in_maps, trace=False, **kwargs):
    nc = get_program()
    return run_bass_kernel_spmd(nc, in_maps, list(range(NCORES)),
                                trace=trace, **kwargs)


def kernel(x, attn_i, w_conv, bn_gamma, bn_beta, bn_mean, bn_var):
    x = np.asarray(x, dtype=np.float32)
    attn_i = np.asarray(attn_i, dtype=np.float32)
    w_conv = np.asarray(w_conv, dtype=np.float32)
    bn_gamma = np.asarray(bn_gamma, dtype=np.float32)
    bn_beta = np.asarray(bn_beta, dtype=np.float32)
    bn_mean = np.asarray(bn_mean, dtype=np.float32)
    bn_var = np.asarray(bn_var, dtype=np.float32)
    in_maps = _shard_inputs(x, attn_i, w_conv, bn_gamma, bn_beta, bn_mean, bn_var)
    res = run_sharded(in_maps)
    return _unshard_output(res.results)
